# revision 27
# baseline (speedup 1.0000x reference)
# DMPNN encoder layer on 8 Trainium2 NeuronCores (Bass/Tile).
#
# Distribution: data-parallel over bonds (16384/core) and atoms (8192/core).
# Gather-sum rounds use windowed dma_gather (int16 indices -> 4 windows of
# 32768 table rows) + dma_scatter_add (CCE add performs the sum over incoming
# bonds; chunking keeps dest indices unique within a scatter call).
# Message tables are bf16 [*, 384] rows; full tables are replicated across
# cores via AllGather between rounds. The reference's first-iteration
# h_message is dead, so only one W_h matmul is needed:
#   m1 = gsum(relu(f_ini @ W_i)); m2 = gsum(m1)
#   h2 = relu(f_ini @ W_i + m2 @ W_h)
#   msgs = gsum_atoms(h2); atoms_h = relu([atom_f, msgs] @ W_o + b_o)
#   out = [segment_mean(atoms_h), global_features]
#
# Pipelining: chunks are half-major; scatter dests are scoped to per-half
# accumulator views so each AllGather window fires as soon as its producer
# half completes (AGm1 after r1-half, m2T transposes after r2-half, AGh2
# after each h2 quarter, AGm0 after each phase-0 quarter).
import numpy as np

B = 131072        # bonds
A = 65536         # atoms
APM = 32          # atoms per molecule
D = 300           # hidden
DP = 384          # padded row (768B bf16, %256B for dma_gather)
F = 147           # bond input features
AF = 133          # atom features
NC = 8            # cores
BS = B // NC      # bond shard
AS = A // NC      # atom shard
MS = 2048 // NC   # molecules per core
WIN = 32768       # rows per window tensor
QS = 4096         # quarter-shard rows (window interleave unit)
NW = B // WIN     # 4 windows
SL = 512          # bonds per matmul slab
HALF = BS // 2    # scatter-dest half (pipeline unit)
HSTR = HALF + 128  # acc rows per half (data + trash/pad)

_CACHE = {}
LAST_RESULTS = None


def _pad128(n):
    return (n + 127) & ~127


def _wrap_idx(flat):
    """[L] -> [128, L/16] int16: idx i at (partition i%16, col i//16),
    replicated across the 8 gpsimd core groups."""
    L = len(flat)
    w = flat.reshape(L // 16, 16).T.astype(np.int16)
    return np.tile(w, (8, 1))


CH = 1024         # dests per matmul-sum chunk


def plan_mm_round(src_by_core, n_dest):
    """Matmul-sum (scatter-free) round plan. src_by_core: [NC, n_dest, 4]
    global source rows. Block boundaries on dest ranges are UNIFORM across
    cores (greedy: extend while every core's entry count stays <= 128 and
    extent <= 128 dests) so the SPMD program is identical on every core; only
    gather-index / destloc VALUES differ. Returns:
      blocks: tuple per (chunk, w) of (c0, c1) tuples  (uniform, hashable)
      gidx[core]: int32 flat gather stream (nblk*128 rows, window-local idx)
      dloc[core]: float32 [128, nblk] destloc columns (-1 = pad)"""
    nchunk = n_dest // CH
    blocks_u = []
    per_core_entries = []
    for c in range(nchunk):
        for w in range(NW):
            percore = []
            for k in range(NC):
                s = src_by_core[k][c * CH:(c + 1) * CH]
                ww = (s % BS) // QS
                loc = (s // BS) * QS + (s % QS)
                dd, jj = np.nonzero(ww == w)
                order = np.argsort(dd, kind="stable")
                percore.append((dd[order], loc[dd[order], jj[order]]))
            cnt = np.zeros((NC, CH), np.int64)
            for k in range(NC):
                np.add.at(cnt[k], percore[k][0], 1)
            ccum = np.cumsum(cnt, axis=1)
            blks = []
            c0 = 0
            while c0 < CH:
                base = ccum[:, c0 - 1] if c0 > 0 else np.zeros(NC, np.int64)
                best = c0 + 1
                e = c0 + 1
                while e <= min(c0 + 128, CH):
                    if (ccum[:, e - 1] - base).max() <= 128:
                        best = e
                        e += 1
                    else:
                        break
                # 32-align boundaries: DVE/Act SBUF accesses need start
                # partitions at multiples of 32. <=32*4 entries always fit.
                best = (best // 32) * 32
                if best <= c0:
                    best = min(c0 + 32, CH)
                blks.append((c0, best))
                c0 = best
            blocks_u.append(tuple(blks))
            per_core_entries.append(percore)

    gidx = [[] for _ in range(NC)]
    dloc = [[] for _ in range(NC)]
    for ci in range(nchunk * NW):
        blks = blocks_u[ci]
        for k in range(NC):
            dd, loc = per_core_entries[ci][k]
            pos = 0
            for (b0, b1) in blks:
                g = np.zeros(128, np.int32)
                dl = np.full(128, -1.0, np.float32)
                n = 0
                while pos < len(dd) and dd[pos] < b1:
                    g[n] = loc[pos]
                    dl[n] = dd[pos] - b0
                    n += 1
                    pos += 1
                assert n <= 128
                gidx[k].append(g)
                dloc[k].append(dl)
            assert pos == len(dd)
    return (nchunk, tuple(blocks_u),
            [np.concatenate(g) for g in gidx],
            [np.stack(d, 1) for d in dloc])


def _to_bf16(x):
    import ml_dtypes
    return np.asarray(x, dtype=ml_dtypes.bfloat16)


def _patch_tile_drain():
    """This walrus build rejects CTRL instructions with >2 sync waits; split
    the TileContext kernel-tail drain's waits into single-wait nops."""
    import concourse.mybir as mybir
    from concourse.tile import TileContext, ScopedClock
    if getattr(TileContext, "_drain_patched", False):
        return

    def _drain_and_barrier(self, tick_clock, wait_clock):
        probe = self.nc.sync.nop()
        wait_clock.add_sem_waits(probe.ins,
                                 ScopedClock({None: tick_clock.global_clock}))
        si = probe.ins.sync_info
        waits = list(si.on_wait) if si is not None else []
        if si is not None:
            si.on_wait = waits[:1]
        for w in waits[1:]:
            n = self.nc.sync.nop()
            if n.ins.sync_info is None:
                n.ins.sync_info = mybir.SyncInfo(on_wait=[w], on_update=[])
            else:
                n.ins.sync_info.on_wait = [w]
        self.nc.sync.drain()
        self.nc.all_engine_barrier()
        assert self.sems is not None
        popped = self.nc._tile_sem_poison_stack.pop()
        assert popped is self._sem_poison
        self.nc.clear_and_free_semaphores(list(self.sems.allocated().values()))
        self.nc.all_engine_barrier()

    TileContext._drain_and_barrier = _drain_and_barrier
    TileContext._drain_patched = True


def _build_program(metaB, metaA):
    import os
    PHASES = int(os.environ.get("KDBG_PHASES", "7"))
    REPEAT = int(os.environ.get("KDBG_REPEAT", "1"))
    NQ = 4
    nchunkB, blocksB = metaB
    nchunkA, blocksA = metaA
    nblkB = sum(len(b) for b in blocksB)
    nblkA = sum(len(b) for b in blocksA)
    import concourse.bacc as bacc
    import concourse.mybir as mybir
    from concourse.tile import TileContext
    from concourse import library_config
    _patch_tile_drain()

    bf16 = mybir.dt.bfloat16
    f32 = mybir.dt.float32
    i16 = mybir.dt.int16
    RELU = mybir.ActivationFunctionType.Relu

    nc = bacc.Bacc("TRN2", target_bir_lowering=False, debug=False,
                   num_devices=NC, num_swdge_queues=NQ)

    f_iniT_sh = nc.dram_tensor("f_iniT_sh", [F, BS], bf16, kind="ExternalInput")
    w_i = nc.dram_tensor("w_i", [F, DP], bf16, kind="ExternalInput")
    w_h = nc.dram_tensor("w_h", [DP, DP], bf16, kind="ExternalInput")
    w_oa = nc.dram_tensor("w_oa", [AF + 1, D], bf16, kind="ExternalInput")
    w_om = nc.dram_tensor("w_om", [DP, D], bf16, kind="ExternalInput")
    atom_fT = nc.dram_tensor("atom_fT", [AF + 1, AS], bf16, kind="ExternalInput")
    seg = nc.dram_tensor("seg", [128, 4], f32, kind="ExternalInput")
    gi_b = nc.dram_tensor("gi_b", [128, nblkB * 8], i16, kind="ExternalInput")
    dl_b = nc.dram_tensor("dl_b", [128, nblkB], f32, kind="ExternalInput")
    gi_a = nc.dram_tensor("gi_a", [128, nblkA * 8], i16, kind="ExternalInput")
    dl_a = nc.dram_tensor("dl_a", [128, nblkA], f32, kind="ExternalInput")
    iota_in = nc.dram_tensor("iota_in", [128, 128], bf16, kind="ExternalInput")

    mols = nc.dram_tensor("mols", [MS, D], f32, kind="ExternalOutput")

    m0_q = [nc.dram_tensor(f"m0_q{w}", [WIN, DP], bf16, kind="Internal",
                           addr_space="Shared") for w in range(NW)]
    m0_rows = nc.dram_tensor("m0_rows", [BS, DP], bf16, kind="Internal")
    ACC_R = 2 * HSTR
    ACC_A = AS + 128
    m1_acc = nc.dram_tensor("m1_acc", [ACC_R, DP], bf16, kind="Internal")
    m2_acc = nc.dram_tensor("m2_acc", [ACC_R, DP], bf16, kind="Internal")
    ms_acc = nc.dram_tensor("ms_acc", [ACC_A, DP], bf16, kind="Internal")
    m1_q = [nc.dram_tensor(f"m1_q{w}", [WIN, DP], bf16, kind="Internal",
                           addr_space="Shared") for w in range(NW)]
    h2_rows = nc.dram_tensor("h2_rows", [BS, DP], bf16, kind="Internal")
    h2_q = [nc.dram_tensor(f"h2_q{w}", [WIN, DP], bf16, kind="Internal",
                           addr_space="Shared") for w in range(NW)]

    rg = [list(range(NC))]

    def ag(src_ap, dst_t):
        nc.gpsimd.collective_compute(
            "AllGather", mybir.AluOpType.bypass,
            ins=[src_ap], outs=[dst_t[:, :]], replica_groups=rg)

    # m1/m2 acc data rows for global quarter w: halves are strided by HSTR
    def acc_quarter(acc, w):
        r0 = (w // 2) * HSTR + (w % 2) * QS
        return acc[r0:r0 + QS, :]

    with TileContext(nc, num_cores=NC) as tc:
        with tc.tile_pool(name="const", bufs=1) as cpool:
            nc.gpsimd.load_library(library_config.mlp)

            wi_a = cpool.tile([128, DP], bf16)
            wi_b = cpool.tile([F - 128, DP], bf16)
            nc.sync.dma_start(wi_a[:], w_i[0:128, :])
            nc.sync.dma_start(wi_b[:], w_i[128:F, :])
            iota_t = cpool.tile([128, 128], bf16)
            nc.sync.dma_start(iota_t[:], iota_in[:, :])
            dlb_t = cpool.tile([128, nblkB], f32)
            nc.sync.dma_start(dlb_t[:], dl_b[:, :])
            dla_t = cpool.tile([128, nblkA], f32)
            nc.sync.dma_start(dla_t[:], dl_a[:, :])

            for _rep in range(REPEAT):
                # ---- phase 0: m0 = relu(f_ini @ W_i); AG#w fires per quarter
                with (
                    tc.tile_pool(name="p0", bufs=3) as pool,
                    tc.tile_pool(name="p0ps", bufs=8, space="PSUM") as pspool,
                ):
                    for s0 in range(0, BS, SL):
                        fa = pool.tile([128, SL], bf16, tag="fa")
                        fb = pool.tile([F - 128, SL], bf16, tag="fb")
                        nc.sync.dma_start(fa[:], f_iniT_sh[0:128, s0:s0 + SL])
                        nc.sync.dma_start(fb[:], f_iniT_sh[128:F, s0:s0 + SL])
                        stage = pool.tile([128, SL // 128, DP], bf16, tag="st")
                        for t in range(SL // 128):
                            ps = pspool.tile([128, DP], f32)
                            nc.tensor.matmul(ps[:], fa[:, t * 128:(t + 1) * 128],
                                             wi_a[:], start=True, stop=False)
                            nc.tensor.matmul(ps[:], fb[:, t * 128:(t + 1) * 128],
                                             wi_b[:], start=False, stop=True)
                            if t % 2 == 0:
                                nc.scalar.activation(stage[:, t, :], ps[:], RELU)
                            else:
                                nc.vector.tensor_scalar_max(stage[:, t, :], ps[:], 0.0)
                        nc.sync.dma_start(
                            m0_rows[s0:s0 + SL, :].rearrange("(s p) d -> p s d", p=128),
                            stage[:])
                        if (s0 + SL) % QS == 0:
                            w = (s0 + SL) // QS - 1
                            ag(m0_rows[w * QS:(w + 1) * QS, :], m0_q[w])

                # ---- matmul-sum round (scatter-free) ----
                # per (chunk, w): one dest-sorted gather; per uniform block:
                # S = (iota == destloc); psum = S^T @ buf_block; exact-width
                # copy into the window partial. Merge 4 partials on DVE and
                # write rows sequentially.
                def mm_round(tables, acc, gi, dlt, nchunk, blocks, hstr,
                             bufs, after_chunk=None):
                    with (
                        tc.tile_pool(name="mmr", bufs=bufs) as pool,
                        tc.tile_pool(name="mmps", bufs=8, space="PSUM") as pspool,
                    ):
                        bcount = 0
                        soff = 0
                        for c in range(nchunk):
                            pw_t = [pool.tile([128, CH // 128, DP], bf16,
                                              tag=f"pw{w}", name=f"pw{w}")
                                    for w in range(NW)]
                            for w in range(NW):
                                blks = blocks[c * NW + w]
                                nb = len(blks)
                                G = nb * 128
                                git = pool.tile([128, G // 16], i16, tag="git")
                                nc.sync.dma_start(
                                    git[:], gi[:, soff // 16:(soff + G) // 16])
                                buf = pool.tile([128, nb, DP], bf16, tag="buf")
                                nc.gpsimd.dma_gather(
                                    buf[:], tables[w][:, :], git[:],
                                    G, G, DP, single_packet=False,
                                    queue_num=w % NQ)
                                for bi, (c0, c1) in enumerate(blks):
                                    W = c1 - c0
                                    S = pool.tile([128, 128], bf16, tag="S")
                                    nc.vector.tensor_scalar(
                                        S[:], iota_t[:],
                                        dlt[:, bcount:bcount + 1], None,
                                        mybir.AluOpType.is_equal)
                                    ps = pspool.tile([128, DP], f32)
                                    nc.tensor.matmul(ps[:], S[:], buf[:, bi, :],
                                                     start=True, stop=True)
                                    # copy psum -> partial in 32-partition
                                    # pieces (DVE/Act partition-range rule)
                                    s_ = 0
                                    while s_ < W:
                                        n32 = min(32, W - s_)
                                        dd0 = c0 + s_
                                        dst = pw_t[w][dd0 % 128:dd0 % 128 + n32,
                                                      dd0 // 128, :]
                                        if (bi + s_ // 32) % 2 == 0:
                                            nc.scalar.activation(
                                                dst, ps[s_:s_ + n32, :],
                                                mybir.ActivationFunctionType.Copy)
                                        else:
                                            nc.vector.tensor_copy(
                                                dst, ps[s_:s_ + n32, :])
                                        s_ += n32
                                    bcount += 1
                                soff += G
                            nc.vector.tensor_add(pw_t[0][:], pw_t[0][:], pw_t[1][:])
                            nc.vector.tensor_add(pw_t[2][:], pw_t[2][:], pw_t[3][:])
                            nc.vector.tensor_add(pw_t[0][:], pw_t[0][:], pw_t[2][:])
                            d0 = c * CH
                            base = (d0 // (BS // 2)) * hstr + (d0 % (BS // 2)) \
                                if hstr else d0
                            nc.sync.dma_start(
                                acc[base:base + CH, :].rearrange(
                                    "(s p) d -> p s d", p=128),
                                pw_t[0][:])
                            if after_chunk is not None:
                                after_chunk(c)

                # ---- round 1 (fires AGm1 per quarter) ----
                def r1_after(c):
                    if PHASES >= 2 and (c + 1) % 4 == 0:
                        q = c // 4
                        ag(acc_quarter(m1_acc, q), m1_q[q])

                if PHASES >= 1:
                    mm_round(m0_q, m1_acc, gi_b, dlb_t, nchunkB, blocksB,
                             HSTR, bufs=2, after_chunk=r1_after)
                if PHASES < 7:
                    with tc.tile_pool(name="dbg", bufs=1) as dpool:
                        dt_ = dpool.tile([128, 2 * D], f32)
                        nc.vector.memset(dt_[:], 0.0)
                        nc.sync.dma_start(
                            mols[:, :].rearrange("(s p) d -> p s d", p=128),
                            dt_[:].rearrange("p (s d) -> p s d", d=D))
                if PHASES >= 4:
                    # ---- round 2 with per-half m2T transposes, then
                    # h2 = relu([f_ini_shard | m2] @ [W_i | W_h]) ----
                    with tc.tile_pool(name="m2t", bufs=1) as mpool:
                        m2T = [mpool.tile([128, BS], bf16, tag=f"m2T{i}", name=f"m2T{i}") for i in range(3)]
                        wh_t = [mpool.tile([128, DP], bf16, tag=f"wh{i}", name=f"wh{i}") for i in range(3)]
                        for kt in range(3):
                            nc.sync.dma_start(wh_t[kt][:], w_h[kt * 128:(kt + 1) * 128, :])

                        def r2_after(c):
                            if (c + 1) % 8 == 0:
                                h = c // 8
                                for ft in range(3):
                                    nc.sync.dma_start_transpose(
                                        m2T[ft][:, h * HALF:(h + 1) * HALF],
                                        m2_acc[h * HSTR:h * HSTR + HALF,
                                               ft * 128:(ft + 1) * 128])

                        mm_round(m1_q, m2_acc, gi_b, dlb_t, nchunkB, blocksB,
                                 HSTR, bufs=2, after_chunk=r2_after)
                        with (
                            tc.tile_pool(name="h2", bufs=3) as pool,
                            tc.tile_pool(name="h2ps", bufs=8, space="PSUM") as pspool,
                        ):
                            for s0 in range(0, BS, SL):
                                fa = pool.tile([128, SL], bf16, tag="fa")
                                fb = pool.tile([F - 128, SL], bf16, tag="fb")
                                nc.sync.dma_start(fa[:], f_iniT_sh[0:128, s0:s0 + SL])
                                nc.sync.dma_start(fb[:], f_iniT_sh[128:F, s0:s0 + SL])
                                stage = pool.tile([128, SL // 128, DP], bf16, tag="st")
                                for t in range(SL // 128):
                                    b0 = s0 + t * 128
                                    ps = pspool.tile([128, DP], f32)
                                    nc.tensor.matmul(ps[:], fa[:, t * 128:(t + 1) * 128],
                                                     wi_a[:], start=True, stop=False)
                                    nc.tensor.matmul(ps[:], fb[:, t * 128:(t + 1) * 128],
                                                     wi_b[:], start=False, stop=False)
                                    for kt in range(3):
                                        nc.tensor.matmul(ps[:], m2T[kt][:, b0:b0 + 128],
                                                         wh_t[kt][:], start=False,
                                                         stop=(kt == 2))
                                    if t % 2 == 0:
                                        nc.scalar.activation(stage[:, t, :], ps[:], RELU)
                                    else:
                                        nc.vector.tensor_scalar_max(stage[:, t, :], ps[:], 0.0)
                                nc.sync.dma_start(
                                    h2_rows[s0:s0 + SL, :].rearrange("(s p) d -> p s d", p=128),
                                    stage[:])
                                if PHASES >= 5 and (s0 + SL) % QS == 0:
                                    w = (s0 + SL) // QS - 1
                                    ag(h2_rows[w * QS:(w + 1) * QS, :], h2_q[w])

                    # ---- atom round ----
                    if PHASES >= 6:
                        mm_round(h2_q, ms_acc, gi_a, dla_t, nchunkA, blocksA,
                                 0, bufs=2)

                    # ---- atoms_h ----
                    with tc.tile_pool(name="atom", bufs=1) as apool:
                        msT = [apool.tile([128, AS], bf16, tag=f"msT{i}", name=f"msT{i}") for i in range(3)]
                        for ft in range(3):
                            nc.sync.dma_start_transpose(
                                msT[ft][:], ms_acc[0:AS, ft * 128:(ft + 1) * 128])
                        afa = apool.tile([128, AS], bf16)
                        afb = apool.tile([AF + 1 - 128, AS], bf16)
                        nc.sync.dma_start(afa[:], atom_fT[0:128, :])
                        nc.sync.dma_start(afb[:], atom_fT[128:AF + 1, :])
                        woa_a = apool.tile([128, D], bf16)
                        woa_b = apool.tile([AF + 1 - 128, D], bf16)
                        nc.sync.dma_start(woa_a[:], w_oa[0:128, :])
                        nc.sync.dma_start(woa_b[:], w_oa[128:AF + 1, :])
                        wom_t = [apool.tile([128, D], bf16, tag=f"wom{i}", name=f"wom{i}") for i in range(3)]
                        for kt in range(3):
                            nc.sync.dma_start(wom_t[kt][:], w_om[kt * 128:(kt + 1) * 128, :])
                        seg_t = apool.tile([128, 4], f32)
                        nc.sync.dma_start(seg_t[:], seg[:])
                        with (
                            tc.tile_pool(name="ah", bufs=4) as pool,
                            tc.tile_pool(name="ahps", bufs=4, space="PSUM") as pspool,
                        ):
                            for at in range(AS // 128):
                                a0 = at * 128
                                ps = pspool.tile([128, D], f32, tag="ps")
                                nc.tensor.matmul(ps[:], afa[:, a0:a0 + 128], woa_a[:],
                                                 start=True, stop=False)
                                nc.tensor.matmul(ps[:], afb[:, a0:a0 + 128], woa_b[:],
                                                 start=False, stop=False)
                                for kt in range(3):
                                    nc.tensor.matmul(ps[:], msT[kt][:, a0:a0 + 128],
                                                     wom_t[kt][:], start=False,
                                                     stop=(kt == 2))
                                ah = pool.tile([128, D], f32, tag="ah")
                                nc.vector.tensor_scalar_max(ah[:], ps[:], 0.0)
                                mp = pspool.tile([4, D], f32, tag="mp")
                                nc.tensor.matmul(mp[:], seg_t[:], ah[:],
                                                 start=True, stop=True)
                                msml = pool.tile([4, D], f32, tag="msml")
                                nc.vector.tensor_copy(msml[:], mp[:])
                                nc.sync.dma_start(mols[at * 4:(at + 1) * 4, :], msml[:])

    nc.compile()
    return nc


def _get_program(metaB, metaA):
    import os
    key = (metaB, metaA,
           os.environ.get("KDBG_PHASES", "7"), os.environ.get("KDBG_REPEAT", "1"))
    if key not in _CACHE:
        _CACHE[key] = _build_program(metaB, metaA)
    return _CACHE[key]


def kernel(atom_features, f_ini_atoms_bonds, global_features, W_i, W_h, W_o, b_o,
           atom_to_incoming_bonds, mapping, atom_to_mol):
    from concourse import bass_utils

    atom_features = np.asarray(atom_features, np.float32)
    f_ini = np.asarray(f_ini_atoms_bonds, np.float32)
    global_features = np.asarray(global_features, np.float32)
    W_i_np = np.asarray(W_i, np.float32)
    W_h_np = np.asarray(W_h, np.float32)
    W_o_np = np.asarray(W_o, np.float32)
    b_o_np = np.asarray(b_o, np.float32)
    a2b = np.asarray(atom_to_incoming_bonds, np.int32)
    mp_idx = np.asarray(mapping, np.int32)

    f_iniT = np.ascontiguousarray(f_ini.T)
    wi_pad = np.zeros((F, DP), np.float32)
    wi_pad[:, :D] = W_i_np
    wh_pad = np.zeros((DP, DP), np.float32)
    wh_pad[:D, :D] = W_h_np
    woa = np.zeros((AF + 1, D), np.float32)
    woa[:AF] = W_o_np[:AF]
    woa[AF] = b_o_np
    wom = np.zeros((DP, D), np.float32)
    wom[:D] = W_o_np[AF:]
    atom_fT_full = np.zeros((AF + 1, A), np.float32)
    atom_fT_full[:AF] = atom_features.T
    atom_fT_full[AF] = 1.0
    seg = np.zeros((128, 4), np.float32)
    for q in range(4):
        seg[q * 32:(q + 1) * 32, q] = 1.0 / APM

    nchunkB, blocksB, gidxB, dlocB = plan_mm_round(
        [mp_idx[c * BS:(c + 1) * BS] for c in range(NC)], BS)
    nchunkA, blocksA, gidxA, dlocA = plan_mm_round(
        [a2b[c * AS:(c + 1) * AS] for c in range(NC)], AS)

    prog = _get_program((nchunkB, blocksB), (nchunkA, blocksA))

    f_iniT_bf = _to_bf16(f_iniT)
    shared = {
        "w_i": _to_bf16(wi_pad),
        "w_h": _to_bf16(wh_pad),
        "w_oa": _to_bf16(woa),
        "w_om": _to_bf16(wom),
        "seg": seg,
    }
    iota_np = np.tile(np.arange(128, dtype=np.float32)[None, :], (128, 1))
    shared["iota_in"] = _to_bf16(iota_np)
    in_maps = []
    for c in range(NC):
        m = dict(shared)
        m["f_iniT_sh"] = np.ascontiguousarray(
            f_iniT_bf[:, c * BS:(c + 1) * BS])
        m["atom_fT"] = _to_bf16(atom_fT_full[:, c * AS:(c + 1) * AS])
        m["gi_b"] = _wrap_idx(gidxB[c])
        m["dl_b"] = dlocB[c]
        m["gi_a"] = _wrap_idx(gidxA[c])
        m["dl_a"] = dlocA[c]
        in_maps.append(m)

    global LAST_RESULTS
    res = bass_utils.run_bass_kernel_spmd(prog, in_maps, core_ids=list(range(NC)))
    LAST_RESULTS = res
    mols = np.concatenate([res.results[c]["mols"] for c in range(NC)], 0)
    return np.concatenate([mols, global_features], 1).astype(np.float32)

